# revision 1
# baseline (speedup 1.0000x reference)
"""Trainium2 Bass kernel for nn_IntergraphInteract (GNN message passing).

Math (reference):
    score_e = Xq[u_e] . W . Xt[v_e] + b         (per edge, E=500k)
    beta_e  = sigmoid(score_e); w_e = exp(score_e)
    norm[v] = eps + sum_{e->v} w_e
    Xt_new[v] = sum_{e->v} w_e*((1-beta)Xq[u_e] + beta*Xt[v]) / norm[v]
    Xt_new[v_cons] = Xq[u_cons]

Restructured:
    Z = Xt @ W^T  (so score_e = Xq[u_e] . Z[v_e])
    a_e = w_e - w_e*beta_e
    Xt_new[v] = (A[v] + s[v]*Xt[v]) / norm[v]
      A[v] = sum a_e Xq[u_e],  s[v] = sum w_e*beta_e,  norm = eps + sum w_e

Sharding: edges sorted by destination v; core c owns v in [2500c, 2500(c+1)).
No collectives needed. Per core, 20 frames of 128 destination nodes; edges of
a frame padded to T_f*128 (T_f = max over cores). Per 128-edge subtile:
  - dma_gather Xq[u] rows and Z[v] rows (1KB each)
  - fused DVE mul+reduce -> score
  - batched exp/sigmoid per frame (ACT)
  - one-hot S[e,j] = (v_rel[e]==j) built by DVE is_equal vs iota
  - segment sums via PE matmuls accumulating in PSUM:
      A_frame += (a*S)^T @ G,   [norm|s]_frame += S^T @ [w|w*beta]
  - final combine per frame on DVE, DMA out
Consensus overwrite applied on host (1000 rows).
"""

import sys
import numpy as np
from ml_dtypes import bfloat16 as ml_bf16

for _p in ("/opt/trn_rl_repo",):
    if _p not in sys.path:
        sys.path.insert(0, _p)

NQ, NT, D, E = 10000, 20000, 256, 500000
NCORES = 8
NT_LOC = NT // NCORES          # 2500
P = 128
NFRAMES = -(-NT_LOC // P)      # 20
NT_PAD = NFRAMES * P           # 2560
NQUEUES = 4
EPS = 1e-10
OOB = 999.0                    # v_rel padding value (matches no iota column)

_PROG_CACHE = {}


def _split_excess_waits(nc, maxw=1):
    """The installed walrus rejects instructions carrying more than `maxw`
    semaphore waits ("Too many sync wait commands"), but this bass/Tile
    version freely emits more. Hoist excess waits onto same-engine NOPs
    inserted immediately before the over-waiting instruction (same-engine
    program order makes this semantically equivalent)."""
    import bass_rust

    for bb in nc.main_func.blocks:
        insts = bb.instructions  # live list object
        i = 0
        while i < len(insts):
            inst = insts[i]
            si = inst.sync_info
            eng = inst.engine
            if (
                si is not None
                and si.on_wait
                and len(si.on_wait) > maxw
                and eng in nc.engines
            ):
                waits = list(si.on_wait)
                keep = waits[-maxw:]
                extra = waits[:-maxw]
                si.on_wait = keep
                pos = i
                for j in range(0, len(extra), maxw):
                    chunk = extra[j : j + maxw]
                    nop = nc.engines[eng].nop(nofuse=True, hint="wait_split").ins
                    cur_list = nc.cur_bb.bb.instructions
                    assert cur_list[-1] is nop
                    cur_list.pop()
                    nop.sync_info = bass_rust.SyncInfo(
                        on_wait=chunk, on_update=[]
                    )
                    insts.insert(pos, nop)
                    pos += 1
                    i += 1
            i += 1


def _install_swdge_queue_lane_patch():
    """Tile round-robins SWDGE completion sems DMASW0..7 ignoring queue_num,
    but the ucode locks each sem to one SWDGE queue. Partition the 8 lanes
    by queue: queue q uses lanes {q, q+4}."""
    import concourse.tile_sem_assignment as tsa

    if getattr(tsa.TileClockTick, "_queue_lane_patched", False):
        return
    orig = tsa.TileClockTick._assign_tick

    def patched(self, inst):
        if (
            inst.engine == tsa.mybir.EngineType.Pool
            and isinstance(inst, tsa.DMAInst)
            and not isinstance(inst, tsa.bass_isa.UserSyncedRemoteDMADescs)
        ):
            q = int(getattr(inst, "queue_num", 0) or 0)
            cnt = getattr(self, "_q_lane_cnt", None)
            if cnt is None:
                cnt = self._q_lane_cnt = {}
            k = cnt.get(q, 0)
            cnt[q] = k + 1
            self.next_sw_dma_idx = (q % 4) + 4 * (k % 2)
        return orig(self, inst)

    tsa.TileClockTick._assign_tick = patched
    tsa.TileClockTick._queue_lane_patched = True


def _build_program(t_list):
    """Build the SPMD bass program. t_list[f] = subtile count of frame f."""
    import concourse.bass as bass
    import concourse.mybir as mybir
    import concourse.tile as tile

    _install_swdge_queue_lane_patch()

    f32 = mybir.dt.float32
    bf16 = mybir.dt.bfloat16
    i16 = mybir.dt.int16
    Alu = mybir.AluOpType
    Act = mybir.ActivationFunctionType

    t_tot = sum(t_list)
    idxc = 8 * t_tot

    nc = bass.Bass(num_swdge_queues=NQUEUES)
    xq = nc.declare_dram_parameter("xq", [NQ, D], f32, False)
    xtT = nc.declare_dram_parameter("xtT", [D, NT_PAD], f32, False)
    xt = nc.declare_dram_parameter("xt", [NT_PAD, D], f32, False)
    wT = nc.declare_dram_parameter("wT", [D, D], f32, False)
    bcol = nc.declare_dram_parameter("bcol", [P, 1], f32, False)
    iota = nc.declare_dram_parameter("iota", [P, P], f32, False)
    uidx = nc.declare_dram_parameter("uidx", [P, idxc], i16, False)
    vrel = nc.declare_dram_parameter("vrel", [P, t_tot], f32, False)
    vrelT = nc.declare_dram_parameter("vrelT", [P, t_tot * P], bf16, False)
    iotap = nc.declare_dram_parameter("iotap", [P, 1], f32, False)
    out = nc.declare_dram_parameter("out", [NT_PAD, D], f32, True)

    from concourse import library_config

    with tile.TileContext(nc) as tc:
        nc.gpsimd.load_library(library_config.mlp)
        with (
            tc.tile_pool(name="const", bufs=1) as const,
            tc.tile_pool(name="g", bufs=2) as gpool,
            tc.tile_pool(name="pr", bufs=3) as spool,
            tc.tile_pool(name="sa", bufs=1) as sapool,
            tc.tile_pool(name="st", bufs=1) as stpool,
            tc.tile_pool(name="ps", bufs=2, space="PSUM") as ppool,
            tc.tile_pool(name="zs", bufs=2, space="PSUM") as zspool,
        ):
            # ---- constants ----
            iota_sb = const.tile([P, P], f32)
            nc.sync.dma_start(out=iota_sb[:], in_=iota[:])
            b_sb = const.tile([P, 1], f32)
            nc.sync.dma_start(out=b_sb[:], in_=bcol[:])
            vrel_sb = const.tile([P, t_tot], f32)
            nc.sync.dma_start(out=vrel_sb[:], in_=vrel[:])
            uidx_sb = const.tile([P, idxc], i16)
            nc.sync.dma_start(out=uidx_sb[:], in_=uidx[:])
            iotap_sb = const.tile([P, 1], f32)
            nc.sync.dma_start(out=iotap_sb[:], in_=iotap[:])
            z_bf = const.tile([P, NFRAMES, D], bf16)

            # ---- phase 1: Z = Xt_loc @ W^T -> resident SBUF bf16 ----
            with (
                tc.tile_pool(name="ph1", bufs=3) as ph1,
                tc.tile_pool(name="ph1p", bufs=2, space="PSUM") as ph1p,
            ):
                wT_sb = ph1.tile([P, 2, D], f32, tag="wT")
                nc.sync.dma_start(out=wT_sb[:, 0, :], in_=wT[0:P, :])
                nc.sync.dma_start(out=wT_sb[:, 1, :], in_=wT[P : 2 * P, :])
                for m in range(NFRAMES):
                    lhs0 = ph1.tile([P, P], f32, tag="lhs0")
                    lhs1 = ph1.tile([P, P], f32, tag="lhs1")
                    nc.sync.dma_start(out=lhs0[:], in_=xtT[0:P, m * P : (m + 1) * P])
                    nc.sync.dma_start(
                        out=lhs1[:], in_=xtT[P : 2 * P, m * P : (m + 1) * P]
                    )
                    zp = ph1p.tile([P, D], f32, tag="zp")
                    nc.tensor.matmul(
                        out=zp[:], lhsT=lhs0[:], rhs=wT_sb[:, 0, :],
                        start=True, stop=False,
                    )
                    nc.tensor.matmul(
                        out=zp[:], lhsT=lhs1[:], rhs=wT_sb[:, 1, :],
                        start=False, stop=True,
                    )
                    nc.vector.tensor_copy(out=z_bf[:, m, :], in_=zp[:])

            # ---- phase 2: edge processing per frame ----
            # one shared register per distinct gather size (a register per
            # call exhausts the Pool register file)
            _nreg_cache = {}

            def nreg(n):
                if n not in _nreg_cache:
                    _nreg_cache[n] = nc.gpsimd.to_reg(n)
                return _nreg_cache[n]

            col0 = 0
            icol0 = 0
            qi = 0
            for f in range(NFRAMES):
                tf = t_list[f]
                G = gpool.tile([P, tf, D], f32, tag="G")
                for t0 in range(0, tf, 8):
                    t1 = min(t0 + 8, tf)
                    nch = (t1 - t0) * P
                    nc.gpsimd.dma_gather(
                        G[:, t0:t1, :],
                        xq[:, :],
                        uidx_sb[:, icol0 + 8 * t0 : icol0 + 8 * t1],
                        nch,
                        nreg(nch),
                        D,
                        queue_num=qi % NQUEUES,
                    )
                    qi += 1

                # S^T (bf16, whole frame): S_T[j, e] = (vrelT[j, e] == j)
                vt = stpool.tile([P, tf * P], bf16, tag="vt")
                nc.sync.dma_start(
                    out=vt[:], in_=vrelT[:, col0 * P : (col0 + tf) * P]
                )
                S_T = stpool.tile([P, tf * P], bf16, tag="S_T")
                nc.vector.tensor_scalar(
                    out=S_T[:],
                    in0=vt[:],
                    scalar1=iotap_sb[:, 0:1],
                    scalar2=None,
                    op0=Alu.is_equal,
                )

                # scores: Zsel = S_T.T @ Z_frame per subtile (PE, bf16),
                # then batched mul+reduce per 4-subtile group (DVE)
                score = gpool.tile([P, tf], f32, tag="score")
                Rall = gpool.tile([P, tf, D + 2], f32, tag="Rall")
                sg_sb = gpool.tile([P, tf], f32, tag="sg")
                a_sb = gpool.tile([P, tf], f32, tag="a")
                for t0 in range(0, tf, 4):
                    t1 = min(t0 + 4, tf)
                    zsel = zspool.tile([P, 4, D], f32, tag="zsel")
                    for t in range(t0, t1):
                        nc.tensor.matmul(
                            out=zsel[:, t - t0, :],
                            lhsT=S_T[:, t * P : (t + 1) * P],
                            rhs=z_bf[:, f, :],
                            start=True,
                            stop=True,
                        )
                    prodall = spool.tile([P, 4, D], f32, tag="prodall")
                    nc.vector.tensor_tensor(
                        out=prodall[:, 0 : t1 - t0, :],
                        in0=G[:, t0:t1, :],
                        in1=zsel[:, 0 : t1 - t0, :],
                        op=Alu.mult,
                    )
                    nc.vector.tensor_reduce(
                        out=score[:, t0:t1],
                        in_=prodall[:, 0 : t1 - t0, :],
                        axis=mybir.AxisListType.X,
                        op=Alu.add,
                    )
                    nc.scalar.activation(
                        Rall[:, t0:t1, D : D + 1],
                        score[:, t0:t1, None],
                        Act.Exp,
                        bias=b_sb[:, 0:1],
                    )
                    nc.scalar.activation(
                        sg_sb[:, t0:t1], score[:, t0:t1], Act.Sigmoid,
                        bias=b_sb[:, 0:1],
                    )
                    nc.vector.tensor_tensor(
                        out=Rall[:, t0:t1, D + 1 : D + 2],
                        in0=Rall[:, t0:t1, D : D + 1],
                        in1=sg_sb[:, t0:t1, None],
                        op=Alu.mult,
                    )
                    nc.vector.tensor_tensor(
                        out=a_sb[:, t0:t1, None],
                        in0=Rall[:, t0:t1, D : D + 1],
                        in1=Rall[:, t0:t1, D + 1 : D + 2],
                        op=Alu.subtract,
                    )


                # one-hot S (fp32, whole frame) for the segment matmul
                S_all = sapool.tile([P, tf, P], f32, tag="S_all")
                nc.vector.tensor_tensor(
                    out=S_all[:],
                    in0=vrel_sb[:, col0 : col0 + tf, None].to_broadcast([P, tf, P]),
                    in1=iota_sb[:, None, :].to_broadcast([P, tf, P]),
                    op=Alu.is_equal,
                )

                # segment sums: rhs = [a*G | w | wb]
                Ans_ps = ppool.tile([P, D + 2], f32, tag="Ans")
                for t in range(tf):
                    nc.scalar.activation(
                        Rall[:, t, 0:D],
                        G[:, t, :],
                        Act.Copy,
                        bias=0.0,
                        scale=a_sb[:, t : t + 1],
                    )
                    nc.tensor.matmul(
                        out=Ans_ps[:],
                        lhsT=S_all[:, t, :],
                        rhs=Rall[:, t, :],
                        start=(t == 0),
                        stop=(t == tf - 1),
                    )

                # combine: out_f = (A + s*Xt_f) / (norm + eps)
                ns_sb = gpool.tile([P, 2], f32, tag="nssb")
                nc.vector.tensor_copy(out=ns_sb[:], in_=Ans_ps[:, D : D + 2])
                xt_f = gpool.tile([P, D], f32, tag="xtf")
                nc.sync.dma_start(out=xt_f[:], in_=xt[f * P : (f + 1) * P, :])
                numer = gpool.tile([P, D], f32, tag="numer")
                nc.vector.tensor_scalar_mul(numer[:], xt_f[:], ns_sb[:, 1:2])
                nc.vector.tensor_tensor(
                    out=numer[:], in0=numer[:], in1=Ans_ps[:, 0:D], op=Alu.add
                )
                normv = gpool.tile([P, 1], f32, tag="normv")
                nc.vector.tensor_scalar_add(normv[:], ns_sb[:, 0:1], EPS)
                recip = gpool.tile([P, 1], f32, tag="recip")
                nc.vector.reciprocal(out=recip[:], in_=normv[:])
                nc.vector.tensor_scalar_mul(numer[:], numer[:], recip[:])
                nc.sync.dma_start(out=out[f * P : (f + 1) * P, :], in_=numer[:])

                col0 += tf
                icol0 += 8 * tf

    _split_excess_waits(nc, maxw=1)
    # Raw Bass skips the Bacc pass that fills .instr bytes for extended-ISA
    # instructions (TTR, library load); without it walrus says "ISA wrong
    # length".
    mybir.codegen_inst_isa_subclasses(nc)
    return nc


def _wrap_idx(arr):
    """int16 gather-index layout: position i -> (partition i%16, col i//16),
    replicated to 128 partitions."""
    a = arr.astype(np.int16).reshape(-1, 16).T  # [16, L/16]
    return np.tile(a, (8, 1))


def _prep(u_idx, v_idx):
    """Sort edges by v, shard per core, frame, pad. Returns per-core arrays
    and the global t_list."""
    order = np.argsort(v_idx, kind="stable")
    vs = v_idx[order].astype(np.int64)
    us = u_idx[order].astype(np.int64)

    # boundaries for every (core, frame): v = 2500c + 128f clipped to core range
    bnds = []
    for c in range(NCORES):
        for f in range(NFRAMES):
            bnds.append(min(NT_LOC * c + P * f, NT_LOC * (c + 1)))
    bnds.append(NT)
    bi = np.searchsorted(vs, np.array(bnds))
    counts = np.diff(bi).reshape(NCORES, NFRAMES)

    t_list = [max(1, int(-(-counts[:, f].max() // P))) for f in range(NFRAMES)]

    cores = []
    for c in range(NCORES):
        u_parts, vl_parts, vr_parts = [], [], []
        for f in range(NFRAMES):
            k = c * NFRAMES + f
            lo, hi = bi[k], bi[k + 1]
            n = hi - lo
            L = t_list[f] * P
            ua = np.zeros(L, np.int64)
            vla = np.zeros(L, np.int64)
            vra = np.full(L, OOB, np.float32)
            ua[:n] = us[lo:hi]
            vla[:n] = vs[lo:hi] - NT_LOC * c
            vra[:n] = (vs[lo:hi] - (NT_LOC * c + P * f)).astype(np.float32)
            u_parts.append(_wrap_idx(ua))
            vl_parts.append(_wrap_idx(vla))
            vr_parts.append(vra.reshape(t_list[f], P).T)
        vr_cat = np.concatenate(vr_parts, axis=1)
        # edge-major v_rel row (subtile-major), replicated to 128 partitions
        vrelT_row = vr_cat.T.reshape(1, -1).astype(np.float32)
        cores.append(
            dict(
                uidx=np.ascontiguousarray(np.concatenate(u_parts, axis=1)),
                vrel=np.ascontiguousarray(vr_cat),
                vrelT=np.ascontiguousarray(
                    np.tile(vrelT_row, (128, 1)).astype(ml_bf16)
                ),
            )
        )
    return cores, t_list


def make_in_maps(inputs):
    """Host preprocessing: full inputs -> per-core in_maps + t_list."""
    Xq = np.asarray(inputs["Xq"], np.float32)
    Xt = np.asarray(inputs["Xt"], np.float32)
    W = np.asarray(inputs["W"], np.float32)
    b = np.asarray(inputs["b"], np.float32)
    u_idx = np.asarray(inputs["u_idx"])
    v_idx = np.asarray(inputs["v_idx"])

    cores, t_list = _prep(u_idx, v_idx)
    wTr = np.ascontiguousarray(W.T)
    bcol = np.full((P, 1), b[0], np.float32)
    iota = np.tile(np.arange(P, dtype=np.float32), (P, 1))

    in_maps = []
    for c in range(NCORES):
        xt_c = np.zeros((NT_PAD, D), np.float32)
        xt_c[:NT_LOC] = Xt[c * NT_LOC : (c + 1) * NT_LOC]
        in_maps.append(
            dict(
                xq=Xq,
                xtT=np.ascontiguousarray(xt_c.T),
                xt=xt_c,
                wT=wTr,
                bcol=bcol,
                iota=iota,
                uidx=cores[c]["uidx"],
                vrel=cores[c]["vrel"],
                vrelT=cores[c]["vrelT"],
                iotap=np.arange(P, dtype=np.float32)[:, None],
            )
        )
    return in_maps, t_list


def kernel(**inputs):
    from concourse.bass_utils import run_bass_kernel_spmd

    in_maps, t_list = make_in_maps(inputs)

    key = tuple(t_list)
    if key not in _PROG_CACHE:
        _PROG_CACHE[key] = _build_program(t_list)
    nc = _PROG_CACHE[key]

    res = run_bass_kernel_spmd(nc, in_maps, list(range(NCORES)))

    out = np.concatenate(
        [np.asarray(res.results[c]["out"])[:NT_LOC] for c in range(NCORES)], axis=0
    )
    # consensus overwrite (host): Xt_new[v_cons] = Xq[u_cons]
    u_cons = np.asarray(inputs["u_cons"])
    v_cons = np.asarray(inputs["v_cons"])
    out[v_cons] = np.asarray(inputs["Xq"], np.float32)[u_cons]
    return out

